# revision 43
# baseline (speedup 1.0000x reference)
"""TRN2 Bass kernel for nn_DirectPolicy (MLP + simplex projection policy head).

Self-contained: accepts FULL inputs, shards batch over 8 NeuronCores
(pure data parallel), returns the FULL (B, 65) output.

v3 design:
  - Host pre-transposes the state into zT [20, R] fp16 (rows: X, Y16, TmT,
    ones (bias fold), zero pad) so the kernel needs NO input transposes.
  - Per 512-row group: L1/L2/L3 fp16 matmuls with biases folded via
    ones-rows; hidden 200 packed as [101, 1024] (two 100-halves side by
    side, row 100 = ones) -> ONE Prelu per layer on ACT; raw [65,512]
    cast to fp16 (DVE) -> 4 PE back-transposes -> batch-major
    bm[p, c, 66-padded] fp16 (2x-mode copies).
  - Per 64-column chunk: softplus(2*raw) via ACT Exp (fp32 scratch) + Ln;
    secant on G(th) = sum_d max(u', th) - 64*th (scale-2 domain, mass 2):
    TT-max (1x, bcast), fp16 fold tree (2x) + fp32 tensor_reduce, fp32
    scalar updates with theta canonical in fp16.
  - Final u = relu(u' - thf)*0.5 (TT-sub + ACT Relu w/ scale); consumption
    C = 0.3*X*ev/(1+ev); per-chunk output DMA from double-buffered fp32
    chunk tiles.
"""
import json
import numpy as np

import concourse.bass as bass
import concourse.mybir as mybir
from concourse.tile import TileContext
from concourse import bass_utils

F32 = mybir.dt.float32
F16 = mybir.dt.float16
AF = mybir.ActivationFunctionType
ALU = mybir.AluOpType
AX = mybir.AxisListType

B = 262144
NCORES = 8
R = B // NCORES            # 32768 rows per core
P = 128                    # partitions
CPT = R // P               # 256 columns per partition (p-major mapping)
NG = CPT // 4              # 64 groups of 512 rows
# uneven projection chunks (in groups): small tail chunks shrink the
# serial projection tail after the last MLP group
CHUNK_GROUPS = [8, 8, 8, 8, 8, 8, 8, 8]
CCHMAX = max(CHUNK_GROUPS) * 4
NITER = 3                  # secant iterations
D = 64
HID = 200
SD = 18                    # state dim
SDP = 20                   # padded: 18 data + ones(bias) + zero
DP = 80                    # bm inner pad (XBAR transpose granularity)


# ---------------------------------------------------------------------------
# walrus workaround: split >1 sem-waits per instruction onto NoOps
def _split_excess_waits(bir: dict) -> int:
    n = 0
    ctr = [0]
    for fn in bir.get("functions", []):
        for blk in fn.get("blocks", []):
            out = []
            for inst in blk.get("instructions", []):
                si = inst.get("sync_info")
                ow = (si or {}).get("on_wait") or []
                cap = 2 if inst.get("opcode") == "EventSemaphore" else 1
                if len(ow) > cap:
                    excess, keep = ow[:-cap], ow[-cap:]
                    for w in excess:
                        ctr[0] += 1
                        out.append({
                            "debug": inst.get("debug", 0),
                            "engine": inst.get("engine", "Unassigned"),
                            "ins": [], "outs": [],
                            "name": f"{inst['name']}-wsplit{ctr[0]}",
                            "opcode": "NoOp",
                            "sync_info": {"on_update": [], "on_wait": [w]},
                        })
                    si["on_wait"] = keep
                    n += len(excess)
                out.append(inst)
            blk["instructions"] = out
    return n


_waitfix_done = False


def _install_waitfix():
    global _waitfix_done
    if _waitfix_done:
        return
    _waitfix_done = True
    orig = bass_utils.compile_bir_kernel

    def patched(bir_json, tmpdir, neff_name="file.neff"):
        if isinstance(bir_json, str):
            bir_json = bir_json.encode()
        bir = json.loads(bir_json)
        if _split_excess_waits(bir):
            bir_json = json.dumps(bir).encode()
        return orig(bir_json, tmpdir, neff_name)

    bass_utils.compile_bir_kernel = patched
    try:
        from concourse import bass2jax
        bass2jax.compile_bir_kernel = patched
    except ImportError:
        pass


# ---------------------------------------------------------------------------
def _build_wpack(W1, b1, W2, b2, W3, b3):
    """Pack all weights (bias rows folded) + identity into one [128, n] f16."""
    cols = {}
    pieces = []
    off = 0

    def put(name, arr):  # arr [p, w]
        nonlocal off
        p, w = arr.shape
        a = np.zeros((P, w), np.float16)
        a[:p] = arr
        pieces.append(a)
        cols[name] = (off, w, p)
        off += w

    w1p = np.zeros((SDP, HID), np.float32)
    w1p[:SD] = W1
    w1p[SD] = b1                          # ones-row bias fold
    put("w1", w1p.astype(np.float16))     # [20, 200]
    w2lo = np.zeros((101, HID), np.float32)
    w2lo[:100] = W2[0:100]
    w2lo[100] = b2
    put("w2lo", w2lo.astype(np.float16))  # [101, 200]
    put("w2hi", W2[100:200].astype(np.float16))
    w3lo = np.zeros((101, DP), np.float32)
    w3lo[:100, :D + 1] = W3[0:100]
    w3lo[100, :D + 1] = b3
    put("w3lo", w3lo.astype(np.float16))  # [101, 80]
    w3hi = np.zeros((100, DP), np.float32)
    w3hi[:, :D + 1] = W3[100:200]
    put("w3hi", w3hi.astype(np.float16))
    esc = np.ones((DP, 1), np.float32)
    esc[:D] = 2.0
    esc[D] = 0.5
    put("escale", esc.astype(np.float16))
    eb = np.ones((DP, 1), np.float32)
    eb[:D] = 1.0
    eb[D] = 0.0
    put("ebias", eb.astype(np.float16))
    return np.concatenate(pieces, axis=1), cols


_CACHE = {}


def _build_program():
    nc = bass.Bass("TRN2")
    zt_d = nc.dram_tensor("ztp", (SDP, R), F16, kind="ExternalInput")
    xc_d = nc.dram_tensor("xc", (P, CPT), F32, kind="ExternalInput")
    wp_d = nc.dram_tensor("wp", _CACHE["wpack"].shape, F16, kind="ExternalInput")
    out_d = nc.dram_tensor("opk", (P, CPT * 65), F32, kind="ExternalOutput")
    cols = _CACHE["wcols"]

    with TileContext(nc) as tc:
        import contextlib
        with contextlib.ExitStack() as ctx:
            ctx.enter_context(nc.allow_low_precision(reason="fp16 datapath"))
            sing = ctx.enter_context(tc.tile_pool(name="sing", bufs=1))
            grp = ctx.enter_context(tc.tile_pool(name="grp", bufs=2))
            st = ctx.enter_context(tc.tile_pool(name="st", bufs=2))
            och = ctx.enter_context(tc.tile_pool(name="och", bufs=2))
            pz1 = ctx.enter_context(tc.tile_pool(name="pz1", bufs=2, space="PSUM"))
            pz2 = ctx.enter_context(tc.tile_pool(name="pz2", bufs=1, space="PSUM"))
            pz3 = ctx.enter_context(tc.tile_pool(name="pz3", bufs=2, space="PSUM"))

            # ---- load inputs ----
            zT = sing.tile([SDP, R], F16)
            for q in range(8):
                s = q * (R // 8)
                nc.sync.dma_start(out=zT[:, s:s + R // 8],
                                  in_=zt_d[:, s:s + R // 8])
            xc = sing.tile([P, CPT], F32)
            nc.sync.dma_start(out=xc, in_=xc_d[:, :])
            wp = sing.tile([P, _CACHE["wpack"].shape[1]], F16)
            nc.sync.dma_start(out=wp, in_=wp_d[:, :])

            def wview(name):
                off, w, p = cols[name]
                return wp[:p, off:off + w]

            w1 = wview("w1")
            w2lo, w2hi = wview("w2lo"), wview("w2hi")
            w3lo, w3hi = wview("w3lo"), wview("w3hi")
            # fp32 per-partition scale/bias for feature-major Exp/Ln
            escale = sing.tile([DP, 1], F32)
            nc.vector.memset(escale, 1.0)
            nc.vector.memset(escale[0:64, :], 2.0)
            nc.vector.memset(escale[64:65, :], 0.5)
            ebias = sing.tile([DP, 1], F32)
            nc.vector.memset(ebias, 1.0)
            nc.vector.memset(ebias[64:65, :], 0.0)

            # ---- batch-major result buffer bm[p, c, 80] fp16 ----
            bm = sing.tile([P, CPT, DP], F16)

            # secant state, sliced per chunk
            th0 = sing.tile([P, CPT], F16)
            th1 = sing.tile([P, CPT], F16)
            g0 = sing.tile([P, CPT], F32)
            nc.vector.memset(th0, 0.0)

            scratch = sing.tile([P, CCHMAX, D], F16)
            escr = sing.tile([P, CCHMAX, D], F32)

            # explicit buffers for hidden activations [101, 1024]
            # (row 100 holds persistent ones for the bias fold)
            NRB = 3
            r1s = [sing.tile([101, 1024], F16, name=f"r1_{i}") for i in range(NRB)]
            r2s = [sing.tile([101, 1024], F16, name=f"r2_{i}") for i in range(NRB)]
            for t in (*r1s, *r2s):
                nc.vector.memset(t, 1.0)

            def mlp_l12(g):
                zt = zT[:, 512 * g:512 * (g + 1)]
                # -- L1: z1[:, 0:512] = hid 0:100, z1[:, 512:1024] = hid 100:200
                z1 = pz1.tile([100, 1024], F32, tag="z1")
                nc.tensor.matmul(z1[:, 0:512], w1[:, 0:100], zt,
                                 start=True, stop=True)
                nc.tensor.matmul(z1[:, 512:1024], w1[:, 100:200], zt,
                                 start=True, stop=True)
                r1 = r1s[g % NRB]
                nc.scalar.activation(r1[0:100, :], z1, AF.Prelu, bias=0.0,
                                     scale=1.0, alpha=0.01)
                r1L = r1[:, 0:512]            # [101, 512] incl ones row
                r1H = r1[0:100, 512:1024]     # [100, 512]

                # -- L2
                z2 = pz2.tile([100, 1024], F32, tag="z2")
                nc.tensor.matmul(z2[:, 0:512], w2lo[:, 0:100], r1L,
                                 start=True, stop=False)
                nc.tensor.matmul(z2[:, 0:512], w2hi[:, 0:100], r1H,
                                 start=False, stop=True)
                nc.tensor.matmul(z2[:, 512:1024], w2lo[:, 100:200], r1L,
                                 start=True, stop=False)
                nc.tensor.matmul(z2[:, 512:1024], w2hi[:, 100:200], r1H,
                                 start=False, stop=True)
                r2 = r2s[g % NRB]
                nc.scalar.activation(r2[0:100, :], z2, AF.Prelu, bias=0.0,
                                     scale=1.0, alpha=0.01)

            def mlp_tail(g):
                # -- L3 (output padded to 80 rows for XBAR DMA transpose)
                raw = pz3.tile([DP, 512], F32, tag="raw")
                r2 = r2s[g % NRB]
                nc.tensor.matmul(raw, w3lo, r2[:, 0:512], start=True, stop=False)
                nc.tensor.matmul(raw, w3hi, r2[0:100, 512:1024],
                                 start=False, stop=True)
                # feature-major softplus: praw rows 0:64 = ln(1+e^(2raw)),
                # row 64 = ln(e^(v/2)) = v/2, rows 65:80 = ln(2) filler
                escrF = grp.tile([DP, 512], F32, tag="escrF")
                nc.scalar.activation(escrF, raw, AF.Exp, bias=0.0, scale=escale)
                praw = grp.tile([DP, 512], F16, tag="praw")
                nc.scalar.activation(praw, escrF, AF.Ln, bias=ebias, scale=1.0)

                # -- XBAR DMA transpose straight into bm[p, c, 0:80]
                for j in range(4):
                    nc.sync.dma_start_transpose(
                        out=bm[:, 4 * g + j, :],
                        in_=praw[:, j * 128:(j + 1) * 128])

            def project_chunk(c0, cch):
                csl = slice(c0, c0 + cch)
                sscr = scratch[:, 0:cch, :]
                sescr = escr[:, 0:cch, :]
                uview = bm[:, csl, 0:D]          # [p, cch, 64] u' values

                t0 = th0[:, csl]
                t1 = th1[:, csl]
                G0 = g0[:, csl]
                # s0 = sum_d u' : fp16 fold tree (2x) + fp32 reduce
                sc1 = sscr[:, :, 0:32]
                nc.vector.tensor_tensor(out=sc1, in0=uview[:, :, 0:32],
                                        in1=uview[:, :, 32:64], op=ALU.add)
                nc.vector.tensor_tensor(out=sscr[:, :, 32:48], in0=sc1[:, :, 0:16],
                                        in1=sc1[:, :, 16:32], op=ALU.add)
                nc.vector.tensor_reduce(out=G0, in_=sscr[:, :, 32:48],
                                        axis=AX.X, op=ALU.add)
                # th1 = (s0 - 2)/64 ; f0 = s0 - 2
                nc.vector.tensor_scalar(t1, G0, 1.0 / 64.0, -2.0 / 64.0,
                                        ALU.mult, ALU.add)
                f0i = st.tile([P, CCHMAX], F32, tag="f1", name="f0i")
                nc.vector.tensor_scalar(f0i[:, 0:cch], G0, -2.0, None, ALU.add)

                def stile(tag, dt=F32):
                    t = st.tile([P, CCHMAX], dt, tag=tag, name=tag)
                    return t[:, 0:cch]

                tprev, tcur, fprev = t0, t1, f0i[:, 0:cch]
                for it in range(NITER):
                    sm = nc.vector
                    tb = bass.AP(tensor=tcur.tensor, offset=tcur.offset,
                                 ap=[tcur.ap[0], tcur.ap[1], [0, D]])
                    # scratch = max(u', th)  (1x: bcast operand)
                    nc.vector.tensor_tensor(out=sscr, in0=uview, in1=tb,
                                            op=ALU.max)
                    # fp16 fold tree (2x) then fp32 reduce of 16 lanes
                    sc1 = sscr[:, :, 0:32]
                    nc.vector.tensor_tensor(out=sc1, in0=sscr[:, :, 0:32],
                                            in1=sscr[:, :, 32:64], op=ALU.add)
                    nc.vector.tensor_tensor(out=sscr[:, :, 32:48],
                                            in0=sc1[:, :, 0:16],
                                            in1=sc1[:, :, 16:32], op=ALU.add)
                    F1 = stile("F1")
                    nc.vector.tensor_reduce(out=F1, in_=sscr[:, :, 32:48],
                                            axis=AX.X, op=ALU.add)
                    # f1 = F1 - 64*th - 2
                    thm = stile("thm")
                    sm.tensor_scalar(thm, tcur, -64.0, -2.0, ALU.mult, ALU.add)
                    f1 = stile("f1")
                    sm.tensor_tensor(out=f1, in0=F1, in1=thm, op=ALU.add)
                    dnum = stile("dnum")
                    sm.tensor_tensor(out=dnum, in0=fprev, in1=f1, op=ALU.subtract)
                    sm.tensor_scalar(dnum, dnum, 1e-20, None, ALU.max)
                    den = stile("den")
                    sm.tensor_tensor(out=den, in0=tcur, in1=tprev, op=ALU.subtract)
                    rdnum = stile("rdnum")
                    nc.vector.reciprocal(rdnum, dnum)
                    rk = stile("rk")
                    sm.tensor_tensor(out=rk, in0=den, in1=rdnum, op=ALU.mult)
                    sm.tensor_scalar(rk, rk, 1.0 / 64.0, 1.0, ALU.max, ALU.min)
                    # th2 = th + f1 * rk
                    step = stile("step")
                    sm.tensor_tensor(out=step, in0=f1, in1=rk, op=ALU.mult)
                    th2 = stile("th2", F16)
                    sm.tensor_tensor(out=th2, in0=tcur, in1=step, op=ALU.add)
                    tprev, tcur, fprev = tcur, th2, f1

                # thf = max(th, 0)
                thf = stile("thf", F16)
                nc.vector.tensor_scalar(thf, tcur, 0.0, None, ALU.max)
                tfb = bass.AP(tensor=thf.tensor, offset=thf.offset,
                              ap=[thf.ap[0], thf.ap[1], [0, D]])
                # scratch = u' - thf, then obc = relu(0.5*scratch) on Pool
                nc.vector.tensor_tensor(out=sscr, in0=uview, in1=tfb,
                                        op=ALU.subtract)
                obc = och.tile([P, CCHMAX, 65], F32, tag="obc")
                nc.scalar.activation(obc[:, 0:cch, 0:D], sscr, AF.Relu,
                                     bias=0.0, scale=0.5)

                # consumption: C = 0.3 * X * ev/(1+ev), ev = e^v
                ev = stile("ev")
                nc.scalar.activation(ev, bm[:, csl, D], AF.Exp, bias=0.0,
                                     scale=2.0)
                evp = stile("evp")
                nc.vector.tensor_scalar(evp, ev, 1.0, None, ALU.add)
                rev = stile("rev")
                nc.vector.reciprocal(rev, evp)
                xs = stile("xs")
                nc.vector.tensor_scalar(xs, xc[:, csl], 0.3, None, ALU.mult)
                xev = stile("xev")
                nc.vector.tensor_tensor(out=xev, in0=xs, in1=ev, op=ALU.mult)
                nc.vector.tensor_tensor(out=obc[:, 0:cch, D], in0=xev, in1=rev,
                                        op=ALU.mult)
                # chunk output DMA
                nc.sync.dma_start(
                    out=out_d[:, c0 * 65:(c0 + cch) * 65],
                    in_=obc[:, 0:cch, :].rearrange("p c d -> p (c d)"))

            gdone = 0
            chunk_bounds = []
            acc = 0
            for cg in CHUNK_GROUPS:
                chunk_bounds.append((acc * 4, cg * 4))
                acc += cg
            assert acc == NG
            ci = 0
            for g in range(NG):
                mlp_l12(g)
                mlp_tail(g)
                c0, cch = chunk_bounds[ci]
                if (g + 1) * 4 == c0 + cch:
                    project_chunk(c0, cch)
                    ci += 1
    return nc


def kernel(X, Y, TmT, W1, b1, W2, b2, W3, b3):
    _install_waitfix()
    X = np.ascontiguousarray(X, np.float32)
    Y = np.ascontiguousarray(Y, np.float32)
    TmT = np.ascontiguousarray(TmT, np.float32)
    if "wpack" not in _CACHE:
        _CACHE["wpack"], _CACHE["wcols"] = _build_wpack(
            np.asarray(W1, np.float32), np.asarray(b1, np.float32),
            np.asarray(W2, np.float32), np.asarray(b2, np.float32),
            np.asarray(W3, np.float32), np.asarray(b3, np.float32))
        _CACHE["nc"] = _build_program()
    nc = _CACHE["nc"]

    # host-side state transpose: zT [20, R] fp16 per core
    # row r of the batch maps to (partition p=r%128... NOTE: kernel's batch
    # mapping is r = c*128 + p? No: reshape(P, CPT) maps r = p*CPT + c.
    # zT column index must match bm column layout: group g covers rows
    # [512g, 512(g+1)) in the order j*128 + p  <->  bm[p, 4g+j].
    zt_full = np.empty((NCORES, SDP, R), np.float16)
    for i in range(NCORES):
        off = i * R
        z = np.concatenate([X[off:off + R], Y[off:off + R],
                            TmT[off:off + R]], axis=1)  # [R, 18]
        # row index within core: r -> (p, c) with r = p*CPT + c
        # zT[:, g*512 + j*128 + p] = z[p*CPT + 4g + j]
        zr = z.reshape(P, CPT, SD)                       # [p, c, 18]
        zr = zr.reshape(P, NG, 4, SD)                    # [p, g, j, 18]
        zr = np.ascontiguousarray(zr.transpose(3, 1, 2, 0))  # [18, g, j, p]
        zt = np.zeros((SDP, R), np.float16)
        zt[:SD] = zr.reshape(SD, R).astype(np.float16)
        zt[SD] = 1.0
        zt_full[i] = zt

    in_maps = []
    for i in range(NCORES):
        off = i * R
        in_maps.append({
            "ztp": zt_full[i],
            "xc": X[off:off + R].reshape(P, CPT),
            "wp": _CACHE["wpack"],
        })
    res = bass_utils.run_bass_kernel_spmd(nc, in_maps, core_ids=list(range(NCORES)))
    out = np.empty((B, 65), np.float32)
    for i in range(NCORES):
        out[i * R:(i + 1) * R] = res.results[i]["opk"].reshape(R, 65)
    return out


# revision 44
# speedup vs baseline: 1.3193x; 1.3193x over previous
"""TRN2 Bass kernel for nn_DirectPolicy (MLP + simplex projection policy head).

Self-contained: accepts FULL inputs, shards batch over 8 NeuronCores
(pure data parallel), returns the FULL (B, 65) output.

v3 design:
  - Host pre-transposes the state into zT [20, R] fp16 (rows: X, Y16, TmT,
    ones (bias fold), zero pad) so the kernel needs NO input transposes.
  - Per 512-row group: L1/L2/L3 fp16 matmuls with biases folded via
    ones-rows; hidden 200 packed as [101, 1024] (two 100-halves side by
    side, row 100 = ones) -> ONE Prelu per layer on ACT; raw [65,512]
    cast to fp16 (DVE) -> 4 PE back-transposes -> batch-major
    bm[p, c, 66-padded] fp16 (2x-mode copies).
  - Per 64-column chunk: softplus(2*raw) via ACT Exp (fp32 scratch) + Ln;
    secant on G(th) = sum_d max(u', th) - 64*th (scale-2 domain, mass 2):
    TT-max (1x, bcast), fp16 fold tree (2x) + fp32 tensor_reduce, fp32
    scalar updates with theta canonical in fp16.
  - Final u = relu(u' - thf)*0.5 (TT-sub + ACT Relu w/ scale); consumption
    C = 0.3*X*ev/(1+ev); per-chunk output DMA from double-buffered fp32
    chunk tiles.
"""
import json
import numpy as np

import concourse.bass as bass
import concourse.mybir as mybir
from concourse.tile import TileContext
from concourse import bass_utils

F32 = mybir.dt.float32
F16 = mybir.dt.float16
AF = mybir.ActivationFunctionType
ALU = mybir.AluOpType
AX = mybir.AxisListType

B = 262144
NCORES = 8
R = B // NCORES            # 32768 rows per core
P = 128                    # partitions
CPT = R // P               # 256 columns per partition (p-major mapping)
NG = CPT // 4              # 64 groups of 512 rows
# uneven projection chunks (in groups): small tail chunks shrink the
# serial projection tail after the last MLP group
CHUNK_GROUPS = [8, 8, 8, 8, 8, 8, 8, 8]
CCHMAX = max(CHUNK_GROUPS) * 4
NITER = 3                  # secant iterations
D = 64
HID = 200
SD = 18                    # state dim
SDP = 20                   # padded: 18 data + ones(bias) + zero
DP = 80                    # bm inner pad (XBAR transpose granularity)


# ---------------------------------------------------------------------------
# walrus workaround: split >1 sem-waits per instruction onto NoOps
def _split_excess_waits(bir: dict) -> int:
    n = 0
    ctr = [0]
    for fn in bir.get("functions", []):
        for blk in fn.get("blocks", []):
            out = []
            for inst in blk.get("instructions", []):
                si = inst.get("sync_info")
                ow = (si or {}).get("on_wait") or []
                cap = 2 if inst.get("opcode") == "EventSemaphore" else 1
                if len(ow) > cap:
                    excess, keep = ow[:-cap], ow[-cap:]
                    for w in excess:
                        ctr[0] += 1
                        out.append({
                            "debug": inst.get("debug", 0),
                            "engine": inst.get("engine", "Unassigned"),
                            "ins": [], "outs": [],
                            "name": f"{inst['name']}-wsplit{ctr[0]}",
                            "opcode": "NoOp",
                            "sync_info": {"on_update": [], "on_wait": [w]},
                        })
                    si["on_wait"] = keep
                    n += len(excess)
                out.append(inst)
            blk["instructions"] = out
    return n


_waitfix_done = False


def _install_waitfix():
    global _waitfix_done
    if _waitfix_done:
        return
    _waitfix_done = True
    orig = bass_utils.compile_bir_kernel

    def patched(bir_json, tmpdir, neff_name="file.neff"):
        if isinstance(bir_json, str):
            bir_json = bir_json.encode()
        bir = json.loads(bir_json)
        if _split_excess_waits(bir):
            bir_json = json.dumps(bir).encode()
        return orig(bir_json, tmpdir, neff_name)

    bass_utils.compile_bir_kernel = patched
    try:
        from concourse import bass2jax
        bass2jax.compile_bir_kernel = patched
    except ImportError:
        pass


# ---------------------------------------------------------------------------
def _build_wpack(W1, b1, W2, b2, W3, b3):
    """Pack all weights (bias rows folded) + identity into one [128, n] f16."""
    cols = {}
    pieces = []
    off = 0

    def put(name, arr):  # arr [p, w]
        nonlocal off
        p, w = arr.shape
        a = np.zeros((P, w), np.float16)
        a[:p] = arr
        pieces.append(a)
        cols[name] = (off, w, p)
        off += w

    w1p = np.zeros((SDP, HID), np.float32)
    w1p[:SD] = W1
    w1p[SD] = b1                          # ones-row bias fold
    put("w1", w1p.astype(np.float16))     # [20, 200]
    w2lo = np.zeros((101, HID), np.float32)
    w2lo[:100] = W2[0:100]
    w2lo[100] = b2
    put("w2lo", w2lo.astype(np.float16))  # [101, 200]
    put("w2hi", W2[100:200].astype(np.float16))
    w3lo = np.zeros((101, DP), np.float32)
    w3lo[:100, :D + 1] = W3[0:100]
    w3lo[100, :D + 1] = b3
    put("w3lo", w3lo.astype(np.float16))  # [101, 80]
    w3hi = np.zeros((100, DP), np.float32)
    w3hi[:, :D + 1] = W3[100:200]
    put("w3hi", w3hi.astype(np.float16))
    esc = np.ones((DP, 1), np.float32)
    esc[:D] = 2.0
    esc[D] = 0.5
    put("escale", esc.astype(np.float16))
    eb = np.ones((DP, 1), np.float32)
    eb[:D] = 1.0
    eb[D] = 0.0
    put("ebias", eb.astype(np.float16))
    return np.concatenate(pieces, axis=1), cols


_CACHE = {}


def _build_program():
    nc = bass.Bass("TRN2")
    zt_d = nc.dram_tensor("ztp", (SDP, R), F16, kind="ExternalInput")
    xc_d = nc.dram_tensor("xc", (P, CPT), F32, kind="ExternalInput")
    wp_d = nc.dram_tensor("wp", _CACHE["wpack"].shape, F16, kind="ExternalInput")
    out_d = nc.dram_tensor("opk", (P, CPT * 65), F32, kind="ExternalOutput")
    cols = _CACHE["wcols"]

    with TileContext(nc) as tc:
        import contextlib
        with contextlib.ExitStack() as ctx:
            ctx.enter_context(nc.allow_low_precision(reason="fp16 datapath"))
            sing = ctx.enter_context(tc.tile_pool(name="sing", bufs=1))
            grp = ctx.enter_context(tc.tile_pool(name="grp", bufs=2))
            st = ctx.enter_context(tc.tile_pool(name="st", bufs=2))
            och = ctx.enter_context(tc.tile_pool(name="och", bufs=2))
            pz1 = ctx.enter_context(tc.tile_pool(name="pz1", bufs=2, space="PSUM"))
            pz2 = ctx.enter_context(tc.tile_pool(name="pz2", bufs=1, space="PSUM"))
            pz3 = ctx.enter_context(tc.tile_pool(name="pz3", bufs=2, space="PSUM"))

            # ---- load inputs ----
            zT = sing.tile([SDP, R], F16)
            for q in range(8):
                s = q * (R // 8)
                nc.sync.dma_start(out=zT[:, s:s + R // 8],
                                  in_=zt_d[:, s:s + R // 8])
            xc = sing.tile([P, CPT], F32)
            nc.sync.dma_start(out=xc, in_=xc_d[:, :])
            wp = sing.tile([P, _CACHE["wpack"].shape[1]], F16)
            nc.sync.dma_start(out=wp, in_=wp_d[:, :])

            def wview(name):
                off, w, p = cols[name]
                return wp[:p, off:off + w]

            w1 = wview("w1")
            w2lo, w2hi = wview("w2lo"), wview("w2hi")
            w3lo, w3hi = wview("w3lo"), wview("w3hi")
            # fp32 per-partition scale/bias for feature-major Exp/Ln
            escale = sing.tile([DP, 1], F32)
            nc.vector.memset(escale, 1.0)
            nc.vector.memset(escale[0:64, :], 2.0)
            nc.vector.memset(escale[64:65, :], 0.5)
            ebias = sing.tile([DP, 1], F32)
            nc.vector.memset(ebias, 1.0)
            nc.vector.memset(ebias[64:65, :], 0.0)

            # ---- batch-major result buffer bm[p, c, 80] fp16 ----
            bm = sing.tile([P, CPT, DP], F16)

            # secant state, sliced per chunk
            th0 = sing.tile([P, CPT], F16)
            th1 = sing.tile([P, CPT], F16)
            g0 = sing.tile([P, CPT], F32)
            nc.vector.memset(th0, 0.0)

            scratch = sing.tile([P, CCHMAX, D], F16)
            escr = sing.tile([P, CCHMAX, D], F32)

            # explicit buffers for hidden activations [101, 1024]
            # (row 100 holds persistent ones for the bias fold)
            NRB = 3
            r1s = [sing.tile([101, 1024], F16, name=f"r1_{i}") for i in range(NRB)]
            r2s = [sing.tile([101, 1024], F16, name=f"r2_{i}") for i in range(NRB)]
            for t in (*r1s, *r2s):
                nc.vector.memset(t, 1.0)

            def mlp_l12(g):
                zt = zT[:, 512 * g:512 * (g + 1)]
                # -- L1: z1[:, 0:512] = hid 0:100, z1[:, 512:1024] = hid 100:200
                z1 = pz1.tile([100, 1024], F32, tag="z1")
                nc.tensor.matmul(z1[:, 0:512], w1[:, 0:100], zt,
                                 start=True, stop=True)
                nc.tensor.matmul(z1[:, 512:1024], w1[:, 100:200], zt,
                                 start=True, stop=True)
                r1 = r1s[g % NRB]
                nc.scalar.activation(r1[0:100, :], z1, AF.Prelu, bias=0.0,
                                     scale=1.0, alpha=0.01)
                r1L = r1[:, 0:512]            # [101, 512] incl ones row
                r1H = r1[0:100, 512:1024]     # [100, 512]

                # -- L2
                z2 = pz2.tile([100, 1024], F32, tag="z2")
                nc.tensor.matmul(z2[:, 0:512], w2lo[:, 0:100], r1L,
                                 start=True, stop=False)
                nc.tensor.matmul(z2[:, 0:512], w2hi[:, 0:100], r1H,
                                 start=False, stop=True)
                nc.tensor.matmul(z2[:, 512:1024], w2lo[:, 100:200], r1L,
                                 start=True, stop=False)
                nc.tensor.matmul(z2[:, 512:1024], w2hi[:, 100:200], r1H,
                                 start=False, stop=True)
                r2 = r2s[g % NRB]
                nc.scalar.activation(r2[0:100, :], z2, AF.Prelu, bias=0.0,
                                     scale=1.0, alpha=0.01)

            def mlp_tail(g):
                # -- L3 (output padded to 80 rows for XBAR DMA transpose)
                raw = pz3.tile([DP, 512], F32, tag="raw")
                r2 = r2s[g % NRB]
                nc.tensor.matmul(raw, w3lo, r2[:, 0:512], start=True, stop=False)
                nc.tensor.matmul(raw, w3hi, r2[0:100, 512:1024],
                                 start=False, stop=True)
                # feature-major softplus: praw rows 0:64 = ln(1+e^(2raw)),
                # row 64 = ln(e^(v/2)) = v/2, rows 65:80 = ln(2) filler
                escrF = grp.tile([DP, 512], F32, tag="escrF")
                nc.scalar.activation(escrF, raw, AF.Exp, bias=0.0, scale=escale)
                praw = grp.tile([DP, 512], F16, tag="praw")
                nc.scalar.activation(praw, escrF, AF.Ln, bias=ebias, scale=1.0)

                # -- XBAR DMA transpose straight into bm[p, 4g:4g+4, 0:80]
                # (3D out: flat row index c*128+p matches in column j*128+p)
                nc.sync.dma_start_transpose(
                    out=bm[:, 4 * g:4 * g + 4, :], in_=praw)

            def project_chunk(c0, cch):
                csl = slice(c0, c0 + cch)
                sscr = scratch[:, 0:cch, :]
                sescr = escr[:, 0:cch, :]
                uview = bm[:, csl, 0:D]          # [p, cch, 64] u' values

                t0 = th0[:, csl]
                t1 = th1[:, csl]
                G0 = g0[:, csl]
                # s0 = sum_d u' : fp16 fold tree (2x) + fp32 reduce
                sc1 = sscr[:, :, 0:32]
                nc.vector.tensor_tensor(out=sc1, in0=uview[:, :, 0:32],
                                        in1=uview[:, :, 32:64], op=ALU.add)
                nc.vector.tensor_tensor(out=sscr[:, :, 32:48], in0=sc1[:, :, 0:16],
                                        in1=sc1[:, :, 16:32], op=ALU.add)
                nc.vector.tensor_reduce(out=G0, in_=sscr[:, :, 32:48],
                                        axis=AX.X, op=ALU.add)
                # th1 = (s0 - 2)/64 ; f0 = s0 - 2
                nc.vector.tensor_scalar(t1, G0, 1.0 / 64.0, -2.0 / 64.0,
                                        ALU.mult, ALU.add)
                f0i = st.tile([P, CCHMAX], F32, tag="f1", name="f0i")
                nc.vector.tensor_scalar(f0i[:, 0:cch], G0, -2.0, None, ALU.add)

                def stile(tag, dt=F32):
                    t = st.tile([P, CCHMAX], dt, tag=tag, name=tag)
                    return t[:, 0:cch]

                tprev, tcur, fprev = t0, t1, f0i[:, 0:cch]
                for it in range(NITER):
                    sm = nc.vector
                    tb = bass.AP(tensor=tcur.tensor, offset=tcur.offset,
                                 ap=[tcur.ap[0], tcur.ap[1], [0, D]])
                    # scratch = max(u', th)  (1x: bcast operand)
                    nc.vector.tensor_tensor(out=sscr, in0=uview, in1=tb,
                                            op=ALU.max)
                    # fp16 fold tree (2x) then fp32 reduce of 16 lanes
                    sc1 = sscr[:, :, 0:32]
                    nc.vector.tensor_tensor(out=sc1, in0=sscr[:, :, 0:32],
                                            in1=sscr[:, :, 32:64], op=ALU.add)
                    nc.vector.tensor_tensor(out=sscr[:, :, 32:48],
                                            in0=sc1[:, :, 0:16],
                                            in1=sc1[:, :, 16:32], op=ALU.add)
                    F1 = stile("F1")
                    nc.vector.tensor_reduce(out=F1, in_=sscr[:, :, 32:48],
                                            axis=AX.X, op=ALU.add)
                    # f1 = F1 - 64*th - 2
                    thm = stile("thm")
                    sm.tensor_scalar(thm, tcur, -64.0, -2.0, ALU.mult, ALU.add)
                    f1 = stile("f1")
                    sm.tensor_tensor(out=f1, in0=F1, in1=thm, op=ALU.add)
                    dnum = stile("dnum")
                    sm.tensor_tensor(out=dnum, in0=fprev, in1=f1, op=ALU.subtract)
                    sm.tensor_scalar(dnum, dnum, 1e-20, None, ALU.max)
                    den = stile("den")
                    sm.tensor_tensor(out=den, in0=tcur, in1=tprev, op=ALU.subtract)
                    rdnum = stile("rdnum")
                    nc.vector.reciprocal(rdnum, dnum)
                    rk = stile("rk")
                    sm.tensor_tensor(out=rk, in0=den, in1=rdnum, op=ALU.mult)
                    sm.tensor_scalar(rk, rk, 1.0 / 64.0, 1.0, ALU.max, ALU.min)
                    # th2 = th + f1 * rk
                    step = stile("step")
                    sm.tensor_tensor(out=step, in0=f1, in1=rk, op=ALU.mult)
                    th2 = stile("th2", F16)
                    sm.tensor_tensor(out=th2, in0=tcur, in1=step, op=ALU.add)
                    tprev, tcur, fprev = tcur, th2, f1

                # thf = max(th, 0)
                thf = stile("thf", F16)
                nc.vector.tensor_scalar(thf, tcur, 0.0, None, ALU.max)
                tfb = bass.AP(tensor=thf.tensor, offset=thf.offset,
                              ap=[thf.ap[0], thf.ap[1], [0, D]])
                # scratch = u' - thf, then obc = relu(0.5*scratch) on Pool
                nc.vector.tensor_tensor(out=sscr, in0=uview, in1=tfb,
                                        op=ALU.subtract)
                obc = och.tile([P, CCHMAX, 65], F32, tag="obc")
                nc.scalar.activation(obc[:, 0:cch, 0:D], sscr, AF.Relu,
                                     bias=0.0, scale=0.5)

                # consumption: C = 0.3 * X * ev/(1+ev), ev = e^v
                ev = stile("ev")
                nc.scalar.activation(ev, bm[:, csl, D], AF.Exp, bias=0.0,
                                     scale=2.0)
                evp = stile("evp")
                nc.vector.tensor_scalar(evp, ev, 1.0, None, ALU.add)
                rev = stile("rev")
                nc.vector.reciprocal(rev, evp)
                xs = stile("xs")
                nc.vector.tensor_scalar(xs, xc[:, csl], 0.3, None, ALU.mult)
                xev = stile("xev")
                nc.vector.tensor_tensor(out=xev, in0=xs, in1=ev, op=ALU.mult)
                nc.vector.tensor_tensor(out=obc[:, 0:cch, D], in0=xev, in1=rev,
                                        op=ALU.mult)
                # chunk output DMA
                nc.sync.dma_start(
                    out=out_d[:, c0 * 65:(c0 + cch) * 65],
                    in_=obc[:, 0:cch, :].rearrange("p c d -> p (c d)"))

            gdone = 0
            chunk_bounds = []
            acc = 0
            for cg in CHUNK_GROUPS:
                chunk_bounds.append((acc * 4, cg * 4))
                acc += cg
            assert acc == NG
            ci = 0
            for g in range(NG):
                mlp_l12(g)
                mlp_tail(g)
                c0, cch = chunk_bounds[ci]
                if (g + 1) * 4 == c0 + cch:
                    project_chunk(c0, cch)
                    ci += 1
    return nc


def kernel(X, Y, TmT, W1, b1, W2, b2, W3, b3):
    _install_waitfix()
    X = np.ascontiguousarray(X, np.float32)
    Y = np.ascontiguousarray(Y, np.float32)
    TmT = np.ascontiguousarray(TmT, np.float32)
    if "wpack" not in _CACHE:
        _CACHE["wpack"], _CACHE["wcols"] = _build_wpack(
            np.asarray(W1, np.float32), np.asarray(b1, np.float32),
            np.asarray(W2, np.float32), np.asarray(b2, np.float32),
            np.asarray(W3, np.float32), np.asarray(b3, np.float32))
        _CACHE["nc"] = _build_program()
    nc = _CACHE["nc"]

    # host-side state transpose: zT [20, R] fp16 per core
    # row r of the batch maps to (partition p=r%128... NOTE: kernel's batch
    # mapping is r = c*128 + p? No: reshape(P, CPT) maps r = p*CPT + c.
    # zT column index must match bm column layout: group g covers rows
    # [512g, 512(g+1)) in the order j*128 + p  <->  bm[p, 4g+j].
    zt_full = np.empty((NCORES, SDP, R), np.float16)
    for i in range(NCORES):
        off = i * R
        z = np.concatenate([X[off:off + R], Y[off:off + R],
                            TmT[off:off + R]], axis=1)  # [R, 18]
        # row index within core: r -> (p, c) with r = p*CPT + c
        # zT[:, g*512 + j*128 + p] = z[p*CPT + 4g + j]
        zr = z.reshape(P, CPT, SD)                       # [p, c, 18]
        zr = zr.reshape(P, NG, 4, SD)                    # [p, g, j, 18]
        zr = np.ascontiguousarray(zr.transpose(3, 1, 2, 0))  # [18, g, j, p]
        zt = np.zeros((SDP, R), np.float16)
        zt[:SD] = zr.reshape(SD, R).astype(np.float16)
        zt[SD] = 1.0
        zt_full[i] = zt

    in_maps = []
    for i in range(NCORES):
        off = i * R
        in_maps.append({
            "ztp": zt_full[i],
            "xc": X[off:off + R].reshape(P, CPT),
            "wp": _CACHE["wpack"],
        })
    res = bass_utils.run_bass_kernel_spmd(nc, in_maps, core_ids=list(range(NCORES)))
    out = np.empty((B, 65), np.float32)
    for i in range(NCORES):
        out[i * R:(i + 1) * R] = res.results[i]["opk"].reshape(R, 65)
    return out
